# revision 1
# baseline (speedup 1.0000x reference)
"""GAT layer kernel for Trainium2 (8 NeuronCores, data-parallel over batch).

Reference computation (per graph b):
    Wh  = atoms @ W                      (N, FO)
    s1  = Wh @ a1 ; s2 = Wh @ a2         (N,)
    e   = leaky_relu(s1[:,None]+s2[None,:], 0.1)
    att = softmax(where(adj>0, e, -9e15), axis=1)
    out = elu(att @ Wh)

On-device formulation (no transcendental ever touches the NxN matrix):
    exp(leaky_relu(s)) = max(e^{s1_i} e^{s2_j}, e^{0.1 s1_i} e^{0.1 s2_j})
and because softmax row-normalizes, any per-row factor cancels, so with
r_i = min(e^{-0.9 s1_i}, 15000) (the clamp is row-uniform, hence exact):
    B_ij = max(v_j, r_i * q_j),  v = e^{s2-5}, q = e^{0.1 s2 - 5}
    att_ij = adj_ij B_ij / sum_j adj_ij B_ij
The 0/1 adjacency multiplies post-"exp" (exact: masked entries contribute 0
to numerator and denominator, equivalent to the reference's -9e15 trick).
The denominator comes free as a ones-column appended to Wh in the
P^T @ [Wh|1] matmul.

Layout: scores are built directly in TRANSPOSED [j,i] form.  The host
wrapper ships the 0/1 adjacency pre-transposed and repacked to fp16
(exact), so the device never touches int32 and never transposes it:
a single fused tensor_scalar (mult+max, two per-partition scalar ptrs
q_j/v_j, split DVE/gpsimd) builds each score row B^T[j,:] =
max(q_j * r, v_j) against an r-broadcast tile — these depend only on
node scores, so they run during the adjacency DMA — and one all-SBUF
fp16 tensor_tensor masks it into the P^T row used as matmul lhsT.
Attention matmuls accumulate h rows for all 8 i-chunks across the jc
loop in two PSUM banks per graph, ic-major so each PSUM region's
accumulation group is contiguous (interleaved groups within a bank
lose all but the last-started group on HW).  The softmax divide folds
into scalar-engine Copy activations with a reciprocal scale pointer.
"""

import numpy as np
from contextlib import ExitStack

import concourse.bass as bass
import concourse.tile as tile
import concourse.mybir as mybir
from concourse.masks import make_identity

dt = mybir.dt
Alu = mybir.AluOpType
Act = mybir.ActivationFunctionType

N = 1024          # nodes per graph
F_IN = 128        # input features
FO = 64           # output features
P = 128           # partitions
NCH = N // P      # 8 node chunks
N_CORES = 8
B_FULL = 64
M_SHIFT = 10.0    # exponent recentering; halves go into v and q


def build_gat(bpc: int, reps: int = 1) -> bass.Bass:
    """Emit the bass program for one core processing `bpc` graphs."""
    nc = bass.Bass()
    atoms = nc.declare_dram_parameter("atoms", [bpc, N, F_IN], dt.float32, isOutput=False)
    # adjacency values are 0/1 — shipped host-repacked as fp16 (exact,
    # 1/2 the HBM traffic, kills the on-device int convert) and
    # host-PRE-TRANSPOSED (adj[g, j, i] = adjacency[g, i, j]), which removes
    # all 512 PE block-transposes + their PSUM staging from the device.
    adj = nc.declare_dram_parameter("adj", [bpc, N, N], dt.float16, isOutput=False)
    wext = nc.declare_dram_parameter("wext", [F_IN, FO + 2], dt.float32, isOutput=False)
    selmat = nc.declare_dram_parameter("selmat", [NCH, NCH * P], dt.float32, isOutput=False)
    out = nc.declare_dram_parameter("out", [bpc, N, FO], dt.float32, isOutput=True)

    with tile.TileContext(nc) as tc, ExitStack() as ctx:
        consts = ctx.enter_context(tc.tile_pool(name="consts", bufs=1))
        # PSUM budget (8 banks x 2KB): h 4 + t 2 + mm 2
        psum = ctx.enter_context(tc.tile_pool(name="psum", bufs=2, space="PSUM"))
        gbuf = ctx.enter_context(tc.tile_pool(name="gbuf", bufs=2))
        cbuf = ctx.enter_context(tc.tile_pool(name="cbuf", bufs=4))
        fbuf = ctx.enter_context(tc.tile_pool(name="fbuf", bufs=2))

        ident_f = consts.tile([P, P], dt.float32, tag="idf")
        make_identity(nc, ident_f)
        ident_b = consts.tile([P, P], dt.float16, tag="idb")
        make_identity(nc, ident_b)
        wext_sb = consts.tile([P, FO + 2], dt.float32, tag="wext")
        nc.gpsimd.dma_start(out=wext_sb, in_=wext[:, :])
        bias_mh = consts.tile([P, 1], dt.float32, tag="bmh")
        nc.vector.memset(bias_mh, -M_SHIFT / 2)
        bias_z = consts.tile([P, 1], dt.float32, tag="bz")
        nc.vector.memset(bias_z, 0.0)
        # sel[:, c*P:(c+1)*P] is all-ones in row c: K=8 matmul with it as
        # stationary broadcasts row c of an [8, 128] tile to all partitions.
        sel_sb = consts.tile([NCH, NCH * P], dt.float16, tag="sel")
        nc.gpsimd.dma_start(out=sel_sb, in_=selmat[:, :])

        def precompute(g):
            # ---------------- per-graph precompute (small) ----------------
            atoms_sb = gbuf.tile([P, NCH, F_IN], dt.float32, tag="atoms", name=f"atoms_{g}")
            nc.sync.dma_start(out=atoms_sb, in_=atoms[g].rearrange("(c p) f -> p c f", p=P))

            # transpose atoms chunks: atT[:, c, :] = [feat, node]
            atT_ps = psum.tile([P, NCH, P], dt.float32, tag="mm", bufs=1, name=f"atT_ps_{g}")
            for c in range(NCH):
                nc.tensor.transpose(atT_ps[:, c, :], atoms_sb[:, c, :], ident_f)
            atT_sb = gbuf.tile([P, NCH, P], dt.float32, tag="atT", name=f"atT_{g}")
            nc.scalar.copy(out=atT_sb, in_=atT_ps)

            # [Wh | s1 | s2] = atoms_chunk @ [W | Wa1 | Wa2]
            whones = gbuf.tile([P, NCH, FO + 1], dt.float16, tag="whones", name=f"whones_{g}")
            nc.vector.memset(whones[:, :, FO:FO + 1], 1.0)
            s12 = gbuf.tile([P, NCH, 2], dt.float32, tag="s12", name=f"s12_{g}")
            for h in range(2):
                whc_ps = psum.tile([P, 4, FO + 2], dt.float32, tag="t", name=f"whc_ps_{g}_{h}")
                for cc in range(4):
                    c = h * 4 + cc
                    nc.tensor.matmul(whc_ps[:, cc, :], lhsT=atT_sb[:, c, :], rhs=wext_sb,
                                     start=True, stop=True)
                nc.scalar.copy(out=whones[:, h * 4:(h + 1) * 4, 0:FO],
                               in_=whc_ps[:, :, 0:FO])
                nc.vector.tensor_copy(out=s12[:, h * 4:(h + 1) * 4, :],
                                      in_=whc_ps[:, :, FO:FO + 2])

            # r_i = min(exp(-0.9 s1), 15000); v = exp(s2-5), q = exp(.1 s2-5)
            rraw = gbuf.tile([P, NCH], dt.float32, tag="rraw", name=f"rraw_{g}")
            nc.scalar.activation(rraw, s12[:, :, 0], Act.Exp, bias=bias_z, scale=-0.9)
            rcols16 = gbuf.tile([P, NCH], dt.float16, tag="rcols16", name=f"rcols16_{g}")
            nc.vector.tensor_scalar(rcols16, rraw, 15000.0, None, Alu.min)
            vcols = gbuf.tile([P, NCH], dt.float32, tag="vcols", name=f"vcols_{g}")
            nc.scalar.activation(vcols, s12[:, :, 1], Act.Exp, bias=bias_mh, scale=1.0)
            qcols = gbuf.tile([P, NCH], dt.float32, tag="qcols", name=f"qcols_{g}")
            nc.scalar.activation(qcols, s12[:, :, 1], Act.Exp, bias=bias_mh, scale=0.1)

            # broadcast r across partitions: rb[p, i] = r_i
            rT_ps = psum.tile([NCH, P], dt.float16, tag="t", name=f"rT_ps_{g}")
            nc.tensor.transpose(rT_ps, rcols16, ident_b)
            rT_sb = gbuf.tile([NCH, P], dt.float16, tag="rT", name=f"rT_{g}")
            nc.vector.tensor_copy(out=rT_sb, in_=rT_ps)
            rb_ps = psum.tile([P, N], dt.float32, tag="mm", bufs=1, name=f"rb_ps_{g}")
            for c in range(NCH):
                nc.tensor.matmul(rb_ps[:, c * P:(c + 1) * P], lhsT=sel_sb[:, c * P:(c + 1) * P],
                                 rhs=rT_sb, start=True, stop=True)
            rb = gbuf.tile([P, N], dt.float16, tag="rb", name=f"rb_{g}")
            nc.scalar.copy(out=rb, in_=rb_ps)

            adjf = gbuf.tile([P, NCH, N], dt.float16, tag="adjf", name=f"adjf_{g}")
            h_ps = [psum.tile([P, 4, P], dt.float32, tag="h", bufs=4, name=f"h_ps_{g}_{t}")
                    for t in range(2)]
            res_g = gbuf.tile([P, NCH, FO], dt.float32, tag="res", name=f"res_{g}")
            return dict(adjf=adjf, whones=whones, vcols=vcols, qcols=qcols, rb=rb,
                        h_ps=h_ps, res=res_g, pms=[], ems=[])

        def adj_load(g, hp, st):
            nc.sync.dma_start(
                out=st["adjf"][:, 2 * hp:2 * hp + 2, :],
                in_=adj[g, 2 * hp * P:(2 * hp + 2) * P, :].rearrange("(c p) j -> p c j", p=P))

        def emit_ems(g, st):
            # B^T rows em[j, i] = max(q_j * r_i, v_j) depend only on the
            # node scores — not the adjacency — so they run during the
            # graph's DMA wait, off the post-DMA critical chain.
            for jc in range(NCH):
                em = cbuf.tile([P, N], dt.float16, tag="em", bufs=16,
                               name=f"em_{g}_{jc}")
                eng = nc.gpsimd if jc % 2 else nc.vector
                eng.tensor_scalar(em, st["rb"], st["qcols"][:, jc:jc + 1],
                                  st["vcols"][:, jc:jc + 1], Alu.mult, Alu.max)
                st["ems"].append(em)

        def jc_step(g, jc, st):
            # masked P^T row: the adjacency arrives pre-transposed, so this
            # is a single all-SBUF fp16 multiply.
            pm = cbuf.tile([P, N], dt.float16, tag="pm", bufs=10, name=f"pm_{g}_{jc}")
            nc.vector.tensor_tensor(pm, st["ems"][jc], st["adjf"][:, jc, :], Alu.mult)
            st["pms"].append(pm)

        def attn_mm(g, st, ic0=0, ic1=NCH):
            # h[i, 0:64] + denom col.  ic-major: each PSUM region's 8-matmul
            # accumulation group is contiguous — interleaved groups within a
            # bank lose all but the last-started group's accumulation on HW.
            for ic in range(ic0, ic1):
                for jc in range(NCH):
                    nc.tensor.matmul(st["h_ps"][ic // 4][:, ic % 4, 0:FO + 1],
                                     lhsT=st["pms"][jc][:, ic * P:(ic + 1) * P],
                                     rhs=st["whones"][:, jc, :],
                                     start=(jc == 0), stop=(jc == NCH - 1))

        def finalize(g, st):
            h_ps, res_g = st["h_ps"], st["res"]
            rec = fbuf.tile([P, NCH], dt.float32, tag="rec", name=f"rec_{g}")
            for t in range(2):
                nc.vector.reciprocal(rec[:, t * 4:(t + 1) * 4], h_ps[t][:, :, FO:FO + 1])
            hdiv = fbuf.tile([P, NCH, FO], dt.float32, tag="hdiv", name=f"hdiv_{g}")
            for t in range(2):
                recB = rec[:, t * 4:(t + 1) * 4, None].broadcast_to([P, 4, FO])
                nc.vector.tensor_tensor(hdiv[:, t * 4:(t + 1) * 4, :],
                                        h_ps[t][:, :, 0:FO], recB, Alu.mult)
            hexp = fbuf.tile([P, NCH, FO], dt.float32, tag="hexp", name=f"hexp_{g}")
            nc.scalar.activation(hexp, hdiv, Act.Exp, bias=bias_z)
            em1 = fbuf.tile([P, NCH, FO], dt.float32, tag="em1", name=f"em1_{g}")
            nc.vector.tensor_scalar(em1, hexp, -1.0, 0.0, Alu.add, Alu.min)
            nc.vector.tensor_tensor(res_g, hdiv, em1, Alu.max)
            nc.scalar.dma_start(out=out[g].rearrange("(c p) f -> p c f", p=P), in_=res_g)

        def start_graph(g):
            st = precompute(g)
            for hp in range(NCH // 2):
                adj_load(g, hp, st)
            return st

        for _rep in range(reps):
            states = {0: start_graph(0)}
            emit_ems(0, states[0])
            done = {}
            for g in range(bpc):
                if g + 1 < bpc:
                    states[g + 1] = start_graph(g + 1)
                st = states.pop(g)
                for jc in range(NCH):
                    jc_step(g, jc, st)
                attn_mm(g, st)
                if g >= 1:
                    finalize(g - 1, done.pop(g - 1))
                done[g] = st
                if g + 1 < bpc:
                    emit_ems(g + 1, states[g + 1])
            finalize(bpc - 1, done.pop(bpc - 1))

    # HW allows at most one sync-wait per Matmult/Ldweights; Tile can emit
    # more.  Run the bacc lowering passes that move extra waits onto
    # ldweights / standalone event-semaphore instructions.
    import bass_rust as _br
    _br.move_matmul_waits_to_ldweights(nc.m)
    _br.generate_event_semaphores(nc)
    return nc


_NC_CACHE: dict[int, bass.Bass] = {}


def _get_nc(bpc: int) -> bass.Bass:
    if bpc not in _NC_CACHE:
        _NC_CACHE[bpc] = build_gat(bpc)
    return _NC_CACHE[bpc]


def _make_wext(W: np.ndarray, a: np.ndarray) -> np.ndarray:
    a1 = a[:FO, :]
    a2 = a[FO:, :]
    return np.concatenate([W, W @ a1, W @ a2], axis=1).astype(np.float32)


def _make_sel() -> np.ndarray:
    sel = np.zeros((NCH, NCH * P), dtype=np.float32)
    for c in range(NCH):
        sel[c, c * P:(c + 1) * P] = 1.0
    return sel


def kernel(atoms_vector: np.ndarray, adjacency: np.ndarray, W: np.ndarray,
           a: np.ndarray) -> np.ndarray:
    from concourse.bass_utils import run_bass_kernel_spmd

    B = atoms_vector.shape[0]
    bpc = B // N_CORES
    wext = _make_wext(W, a)
    sel = _make_sel()

    nc = _get_nc(bpc)
    in_maps = []
    for i in range(N_CORES):
        sl = slice(i * bpc, (i + 1) * bpc)
        in_maps.append({
            "atoms": np.ascontiguousarray(atoms_vector[sl]).astype(np.float32, copy=False),
            "adj": np.ascontiguousarray(
                adjacency[sl].transpose(0, 2, 1)).astype(np.float16),
            "wext": wext,
            "selmat": sel,
        })
    res = run_bass_kernel_spmd(nc, in_maps, list(range(N_CORES)))
    return np.concatenate([res.results[i]["out"] for i in range(N_CORES)], axis=0)

